# revision 5
# baseline (speedup 1.0000x reference)
"""Multi-head attention forward (B=2, S=2048, E=1024, H=16, D=64) on 8 TRN2
NeuronCores, tensor-parallel across heads (2 heads/core).

Per core: QKV^T projection with X^T streamed as the moving operand, attention
computed in the S^T/attn^T orientation (softmax denominator obtained by
appending a ones column to V in the PV matmul), out-projection of the core's
128 embed dims giving a partial [4096, 1024] output. Host sums the 8 partials
and adds the output bias.
"""

import os
from contextlib import ExitStack

import numpy as np

import concourse.bass as bass
import concourse.mybir as mybir
import concourse.tile as tile
from concourse import bacc
from concourse.masks import make_identity

# ---- problem constants (hardcoded per contract) ----
B, S, E, H, D = 2, 2048, 1024, 16, 64
P = 128                      # partitions
R = B * S                    # 4096 flattened rows
KO = E // P                  # 8 contraction chunks over E
NKC = S // P                 # 16 key chunks per sequence
HC = 2                       # heads per core
NCORES = 8
RB = 512                     # row block for the QKV projection

# matmul-input mode: 'bf16' (cast inputs to bf16), 'f32r' (fp32 data, fast
# float32r matmuls), 'f32' (exact fp32, 4x slower PE)
MM_MODE = os.environ.get("MHA_MM_MODE", "f32r")
QB_OVERRIDE = int(os.environ.get("MHA_QB", "0"))        # 0 = mode default
# Emission order of the two heads' score matmuls: sequential per head
# (PACK=0) measured slightly faster on HW twice (LDWEIGHTS pull-ahead works
# better with a constant stationary base partition); model-neutral.
PACK_SCORES = bool(int(os.environ.get("MHA_PACK", "0")))
ES_BUFS = int(os.environ.get("MHA_ES_BUFS", "14"))
KCG = int(os.environ.get("MHA_KCG", "2"))               # kc per exp group
SC_BUFS = int(os.environ.get("MHA_SC_BUFS", "2"))
PQ_BUFS = int(os.environ.get("MHA_PQ_BUFS", "2"))
# exp offload: EXP_NG of the NG kc-groups per (qb,h) computed as a one-op
# Schraudolph fast-exp (int32-domain bitcast) on EXP_ENG instead of true exp
# on the ACT engine. Softmax tolerates the ~3% sawtooth on a slice of keys
# (verified 6e-3 end-to-end vs the 2e-2 gate).
EXP_NG = int(os.environ.get("MHA_EXP_NG", "2"))
EXP_ENG = os.environ.get("MHA_EXP_ENG", "dve")         # pool|dve|none
Y_ENG = os.environ.get("MHA_Y_ENG", "dve")
YDT = os.environ.get("MHA_YDT", "bf16")                  # bf16 halves y DMA             # pool|dve: psum->sbuf
PRELOAD = bool(int(os.environ.get("MHA_PRELOAD", "1")))
# DP mode: 2-way batch data-parallel x 4-head tensor-parallel. Each core
# gets ONE batch (X half: 4 MiB bf16, resident in SBUF) and 4 heads
# processed as two head-pair passes; partial outputs accumulate in SBUF so
# only 4 MiB bf16 per core goes back. Halves chip HBM traffic vs pure TP.
DP = bool(int(os.environ.get("MHA_DP", "1")))
# emit out-proj of qb after scores+exp of qb+1: PE fills the
# recip->broadcast->normalize chain latency with useful work
LOOKAHEAD = bool(int(os.environ.get("MHA_LOOKAHEAD", "0")))

FP32 = mybir.dt.float32
INT32 = mybir.dt.int32
INT16 = mybir.dt.int16
FP16 = mybir.dt.float16
EXP = mybir.ActivationFunctionType.Exp
IDENT = mybir.ActivationFunctionType.Identity
# exp(s_raw*0.125) ~ bitcast(int16(s_raw*SCH_A + SCH_B)) — fp16 Schraudolph
# (est is fp16; the f32r BIR verifier rejects non-rounding producers, fp16
# has no such rule). c=0.0434609 centres the sawtooth.
SCH_A = 0.125 * (2.0 ** 10) / float(np.log(2.0))
SCH_B = (15.0 - 0.0434609) * (2.0 ** 10)


def _mode_params(mm_mode):
    if mm_mode == "bf16":
        dt, qb = mybir.dt.bfloat16, 512
    elif mm_mode == "fp16":
        # fp16 streams the PE at the same 1 cycle/row as bf16 (no 32-bit
        # moving-operand penalty that f32r pays on HW) with 10 mantissa bits
        dt, qb = FP16, 512
    elif mm_mode == "f32r":
        dt, qb = mybir.dt.float32r, 512
    elif mm_mode == "f32":
        dt, qb = FP32, 256
    else:
        raise ValueError(mm_mode)
    return dt, (QB_OVERRIDE or qb)


def _xdt(mm_mode):
    """dtype for the X / QKV-weight operands (16-bit halves the dominant X
    DMA; QKV-phase is DMA-gated at fp32). Verified 7.8e-3 end-to-end."""
    if mm_mode == "fp16":
        return FP16
    if os.environ.get("MHA_XDT", "bf16") == "bf16" and mm_mode != "f32":
        return mybir.dt.bfloat16
    return _mode_params(mm_mode)[0]


def build_kernel(tc, xt, wqkv, bqkv, wout, y, sdt, QB, mm_mode, ctx):
    ydt = y.dtype
    nc = tc.nc
    NQB = S // QB
    NRB = S // RB            # row blocks per batch

    # float32r can only be produced by rounding-capable engine ops (ACT/DVE
    # outputs) or DMA of host-pre-rounded data; memset/affine_select cannot.
    # The transpose path therefore stays plain fp32 in f32r mode.
    vdt = FP32 if sdt == mybir.dt.float32r else sdt
    edt = FP16 if sdt == mybir.dt.float32r else sdt   # est / V dtype

    def mm(ap):
        return ap

    const = ctx.enter_context(tc.tile_pool(name="const", bufs=1))
    # PSUM budget: 8 banks = scores 2x2 (KCG banks per tile) + qkv/transpose
    # 2x1 + pa/out-proj shared 2x1.
    ps_sc = ctx.enter_context(tc.tile_pool(name="ps_sc", bufs=SC_BUFS,
                                           space="PSUM"))
    ps_q = ctx.enter_context(tc.tile_pool(name="ps_q", bufs=PQ_BUFS, space="PSUM"))
    ps_pa = ctx.enter_context(tc.tile_pool(name="ps_pa", bufs=2, space="PSUM"))

    xdt = _xdt(mm_mode)
    xt_pool = ctx.enter_context(tc.tile_pool(name="xtp", bufs=2))
    xt_r = xt.rearrange("(ko p) r -> p ko r", p=P)

    wq_sb = const.tile([P, KO, 3 * P], xdt)
    wq_r = wqkv.rearrange("(ko p) m -> p ko m", p=P)
    xt_first = None
    if PRELOAD:
        # Interleave the first row block's X chunks with the QKV weight
        # chunks so the first matmul (needs wq[ko0]+xt[ko0] only) starts
        # after ~2 DMAs instead of after every startup DMA.
        xt_first = xt_pool.tile([P, KO, RB], xdt, tag="xt")
        for ko in range(KO):
            nc.sync.dma_start(wq_sb[:, ko, :], wq_r[:, ko, :])
            nc.sync.dma_start(xt_first[:, ko, :], xt_r[:, ko, 0:RB])
    else:
        for ko in range(KO):
            nc.sync.dma_start(wq_sb[:, ko, :], wq_r[:, ko, :])
    bq_sb = const.tile([P, 3], FP32)
    nc.sync.dma_start(bq_sb, bqkv.rearrange("(m p) -> p m", p=P))
    wo_sb = const.tile([P, E], sdt)
    nc.sync.dma_start(wo_sb, wout)
    ident = const.tile([P, P], vdt)
    make_identity(nc, ident)

    qt = const.tile([P, B, S], sdt)       # Q^T  [2h*64, b, s]
    kt = const.tile([P, B, S], sdt)       # K^T
    vt = const.tile([P, B, S], vdt)       # V^T
    v1 = const.tile([P, B, HC, NKC, D + 1], edt)  # V natural + ones col
    attnT = const.tile([P, B, S], sdt)    # unnormalized-then-normalized attn^T

    ones_col = const.tile([P, 1], FP32)
    nc.vector.memset(ones_col, 1.0)
    nc.vector.tensor_copy(v1[:, :, :, :, D:],
                          ones_col.to_broadcast((P, B, HC, NKC, 1)))

    exps_pool = ctx.enter_context(tc.tile_pool(name="exps", bufs=ES_BUFS))
    rc_pool = ctx.enter_context(tc.tile_pool(name="rc", bufs=2))
    bc_pool = ctx.enter_context(tc.tile_pool(name="bc", bufs=2))
    y_pool = ctx.enter_context(tc.tile_pool(name="yp", bufs=3))

    NG = NKC // KCG

    exp_eng = {"pool": nc.gpsimd, "dve": nc.vector}.get(EXP_ENG)

    # deferred out-proj emissions (lookahead software pipelining)
    pending = []

    def flush_pending():
        for f in pending:
            f()
        pending.clear()

    for b in range(B):
        # ---- QKV^T projection for batch b, V transposes fused in ----
        for rbi in range(NRB):
            rb = b * NRB + rbi
            col = rbi * RB
            if rb == 0 and xt_first is not None:
                xt_t = xt_first
            else:
                xt_t = xt_pool.tile([P, KO, RB], xdt, tag="xt")
                for ko in range(KO):
                    nc.sync.dma_start(xt_t[:, ko, :],
                                      xt_r[:, ko, rb * RB:(rb + 1) * RB])
            if rbi == 0:
                flush_pending()
            for m, dest in enumerate((qt, kt, vt)):
                pst = ps_q.tile([P, RB], FP32, tag="pq", name=f"ps_qkv_{rb}_{m}")
                for ko in range(KO):
                    nc.tensor.matmul(
                        pst, mm(wq_sb[:, ko, m * P:(m + 1) * P]),
                        mm(xt_t[:, ko, :]),
                        start=(ko == 0), stop=(ko == KO - 1))
                nc.vector.tensor_scalar_add(dest[:, b, col:col + RB], pst,
                                            bq_sb[:, m:m + 1])
            # V natural via PE transpose for this row block's kc chunks.
            # One full [128,128] transpose covers both heads (h0 in output
            # cols 0:64, h1 in 64:128).
            for kci in range(RB // P):
                kc = (col // P) + kci
                pst = ps_q.tile([P, P], vdt, tag="pq", name=f"ps_tr_{b}_{kc}")
                nc.tensor.transpose(
                    pst, vt[:, b, kc * P:(kc + 1) * P], ident)
                for h in range(HC):
                    nc.vector.tensor_copy(v1[:, b, h, kc, :D],
                                          pst[:, h * D:(h + 1) * D])

        # ---- attention + out-projection for batch b ----
        # kc-groups of KCG: exp (ScalarE) of one group overlaps scores/PV
        # matmuls of neighbouring groups on the PE.
        for qb in range(NQB):
            pa = {}
            for h in range(HC):
                pa[h] = ps_pa.tile([P, QB], FP32, tag="pa",
                                   name=f"pa_{b}_{qb}_{h}")
            for g in range(NG):
                est = {}
                pst = {}
                for h in range(HC):
                    est[h] = exps_pool.tile([P, KCG, QB], edt, tag="es",
                                            name=f"es_{b}_{qb}_{g}_{h}")
                    pst[h] = ps_sc.tile([P, KCG, QB], FP32, tag="sc",
                                        name=f"ps_sc_{b}_{qb}_{g}_{h}")
                # scores, heads interleaved per kc (disjoint 64-row PE
                # groups run concurrently); one multi-bank exp per head.
                if PACK_SCORES:
                    jh = [(j, h) for j in range(KCG) for h in range(HC)]
                else:
                    jh = [(j, h) for h in range(HC) for j in range(KCG)]
                for j, h in jh:
                    kc = g * KCG + j
                    nc.tensor.matmul(
                        pst[h][:, j, :],
                        mm(kt[h * D:(h + 1) * D, b, kc * P:(kc + 1) * P]),
                        mm(qt[h * D:(h + 1) * D, b, qb * QB:(qb + 1) * QB]),
                        start=True, stop=True)
                for h in range(HC):
                    # offload only the LAST group(s): PE reaches their PV
                    # matmuls late in the qb, so the offload engine's queue
                    # has slack (verified 6e-3 end-to-end with a fixed
                    # approx key block)
                    if exp_eng is not None and g >= NG - EXP_NG:
                        exp_eng.tensor_scalar(
                            est[h].bitcast(INT16), pst[h], SCH_A, SCH_B,
                            mybir.AluOpType.mult, mybir.AluOpType.add)
                    else:
                        nc.scalar.activation(est[h], pst[h], EXP, scale=0.125)
                if g == 1:
                    # previous qb's out-proj fills the rollover latency
                    flush_pending()
                for h in range(HC):
                    for j in range(KCG):
                        kc = g * KCG + j
                        nc.tensor.matmul(
                            pa[h][:D + 1, :], mm(v1[:, b, h, kc, :]),
                            mm(est[h][:, j, :]),
                            start=(kc == 0), stop=(kc == NKC - 1),
                            skip_group_check=True)
            y_eng = {"pool": nc.gpsimd, "dve": nc.vector}[Y_ENG]
            for h in range(HC):
                rc = rc_pool.tile([1, QB], FP32, tag="rc",
                                  name=f"rc_{b}_{qb}_{h}")
                nc.vector.reciprocal(rc, pa[h][D:D + 1, :])
                bc = bc_pool.tile([D, QB], FP32, tag="bc",
                                  name=f"bc_{b}_{qb}_{h}")
                nc.gpsimd.partition_broadcast(bc, rc)
                nc.vector.tensor_tensor(
                    attnT[h * D:(h + 1) * D, b, qb * QB:(qb + 1) * QB],
                    pa[h][:D, :], bc, mybir.AluOpType.mult)

            def emit_outproj(b=b, qb=qb):
                for qc in range(QB // P):
                    q0 = qb * QB + qc * P
                    yt = y_pool.tile([P, E], ydt, tag="yt",
                                     name=f"yt_{b}_{qb}_{qc}")
                    for nh in range(2):
                        # with lookahead the psum comes from the QKV pool
                        # (pa bufs are already claimed by the next qb)
                        Y_POOL = os.environ.get("MHA_Y_POOL", "pa")
                        pool = {"pq": ps_q, "pa": ps_pa}[Y_POOL]
                        pst = pool.tile([P, 512], FP32, tag=Y_POOL,
                                        name=f"ps_y_{b}_{qb}_{qc}_{nh}")
                        nc.tensor.matmul(
                            pst, mm(attnT[:, b, q0:q0 + P]),
                            mm(wo_sb[:, nh * 512:(nh + 1) * 512]),
                            start=True, stop=True)
                        y_eng.tensor_copy(yt[:, nh * 512:(nh + 1) * 512], pst)
                    nc.sync.dma_start(y[b * S + q0: b * S + q0 + P, :], yt)

            if LOOKAHEAD:
                pending.append(emit_outproj)
            else:
                emit_outproj()
    flush_pending()


def build_kernel_dp(tc, xt, wqkv, bqkv, wout, y, sdt, QB, mm_mode, ctx):
    """DP variant: the outer `b` loop runs over HEAD-PAIR PASSES of the
    core's single batch. Same engine structure as build_kernel."""
    ydt = y.dtype
    nc = tc.nc
    NQB = S // QB
    NRB = S // RB
    NYC = S // P

    # vt only feeds the PE transpose; fp16 halves the transpose cost
    # (1.0 vs fp32's 2.0 cycles/row) and is precision-equivalent to f32r
    vdt = FP16 if sdt == mybir.dt.float32r else sdt
    edt = FP16 if sdt == mybir.dt.float32r else sdt

    def mm(ap):
        return ap

    const = ctx.enter_context(tc.tile_pool(name="const", bufs=1))
    ps_sc = ctx.enter_context(tc.tile_pool(name="ps_sc", bufs=SC_BUFS,
                                           space="PSUM"))
    ps_q = ctx.enter_context(tc.tile_pool(name="ps_q", bufs=PQ_BUFS,
                                          space="PSUM"))
    ps_pa = ctx.enter_context(tc.tile_pool(name="ps_pa", bufs=int(os.environ.get("MHA_PA_BUFS", "2")), space="PSUM"))

    xdt = _xdt(mm_mode)
    xt_r = xt.rearrange("(ko p) r -> p ko r", p=P)

    wq_sb = const.tile([P, KO, B, 3 * P], xdt)
    wq_r = wqkv.rearrange("(ko p) (b m) -> p ko b m", p=P, b=B)
    # X resident in SBUF for both passes (read from HBM exactly once).
    # Startup order: the first QKV matmul chain needs pass-0 weights (all ko)
    # plus only the first RB columns of X — issue those first so the PE
    # starts after ~1.8 MiB of DMA instead of the full 5.5 MiB preamble.
    # Two HW DGE queues (SP + ACT; ACT is idle until the first exp) halve the
    # preamble: SP carries pass-0 weights + even X chunks, ACT odd X chunks +
    # pass-1 weights.
    xt_sb = const.tile([P, KO, S], xdt)
    # bias first: the first QKV bias-add (which frees the PSUM banks) waits
    # on it, and it's only 3 KB
    bq_sb = const.tile([P, B, 3], FP32)
    nc.sync.dma_start(bq_sb, bqkv.rearrange("(b m p) -> p b m", p=P, b=B))
    for ko in range(KO):
        nc.sync.dma_start(wq_sb[:, ko, 0, :], wq_r[:, ko, 0, :])
        nc.scalar.dma_start(xt_sb[:, ko, 0:RB], xt_r[:, ko, 0:RB])
    for rbi in range(1, NRB):
        for ko in range(KO):
            eng = nc.sync if ko % 2 == 0 else nc.scalar
            eng.dma_start(xt_sb[:, ko, rbi * RB:(rbi + 1) * RB],
                          xt_r[:, ko, rbi * RB:(rbi + 1) * RB])
    wo_sb = const.tile([P, B, E], sdt)
    for ko in range(KO):
        nc.scalar.dma_start(wq_sb[:, ko, 1, :], wq_r[:, ko, 1, :])
    # out-proj weights last: first consumed ~90 us in
    nc.sync.dma_start(wo_sb, wout.rearrange("(b p) e -> p b e", p=P))
    ident = const.tile([P, P], vdt)
    make_identity(nc, ident)

    qt = const.tile([P, B, S], sdt)
    kt = const.tile([P, B, S], sdt)
    vt = const.tile([P, B, S], vdt)
    v1 = const.tile([P, B, HC, NKC, D + 1], edt)
    attnT = const.tile([P, B, S], sdt)
    y_acc = const.tile([P, NYC, E], ydt)   # pass-0 partial, summed in pass 1

    ones_col = const.tile([P, 1], FP32)
    nc.vector.memset(ones_col, 1.0)
    nc.vector.tensor_copy(v1[:, :, :, :, D:],
                          ones_col.to_broadcast((P, B, HC, NKC, 1)))

    exps_pool = ctx.enter_context(tc.tile_pool(name="exps", bufs=ES_BUFS))
    rc_pool = ctx.enter_context(tc.tile_pool(name="rc", bufs=2))
    bc_pool = ctx.enter_context(tc.tile_pool(name="bc", bufs=2))
    y_pool = ctx.enter_context(tc.tile_pool(name="yp", bufs=3))

    NG = NKC // KCG
    exp_eng = {"pool": nc.gpsimd, "dve": nc.vector}.get(EXP_ENG)

    pending = []

    def flush_pending():
        for f in pending:
            f()
        pending.clear()

    for b in range(B):            # b = head-pair pass
        for rbi in range(NRB):
            col = rbi * RB
            if rbi == 0:
                flush_pending()
            for m, dest in enumerate((qt, kt, vt)):
                pst = ps_q.tile([P, RB], FP32, tag="pq",
                                name=f"ps_qkv_{b}_{rbi}_{m}")
                for ko in range(KO):
                    nc.tensor.matmul(
                        pst, mm(wq_sb[:, ko, b, m * P:(m + 1) * P]),
                        mm(xt_sb[:, ko, col:col + RB]),
                        start=(ko == 0), stop=(ko == KO - 1))
                nc.vector.tensor_scalar_add(dest[:, b, col:col + RB], pst,
                                            bq_sb[:, b, m:m + 1])
            for kci in range(RB // P):
                kc = (col // P) + kci
                pst = ps_q.tile([P, P], vdt, tag="pq", name=f"ps_tr_{b}_{kc}")
                nc.tensor.transpose(
                    pst, vt[:, b, kc * P:(kc + 1) * P], ident)
                for h in range(HC):
                    nc.vector.tensor_copy(v1[:, b, h, kc, :D],
                                          pst[:, h * D:(h + 1) * D])
        for qb in range(NQB):
            pa = {}
            for h in range(HC):
                pa[h] = ps_pa.tile([P, QB], FP32, tag="pa",
                                   name=f"pa_{b}_{qb}_{h}")
            est_hist = []
            for g in range(NG):
                est = {}
                pst = {}
                for h in range(HC):
                    est[h] = exps_pool.tile([P, KCG, QB], edt, tag="es",
                                            name=f"es_{b}_{qb}_{g}_{h}")
                    pst[h] = ps_sc.tile([P, KCG, QB], FP32, tag="sc",
                                        name=f"ps_sc_{b}_{qb}_{g}_{h}")
                if PACK_SCORES:
                    jh = [(j, h) for j in range(KCG) for h in range(HC)]
                else:
                    jh = [(j, h) for h in range(HC) for j in range(KCG)]
                for j, h in jh:
                    kc = g * KCG + j
                    nc.tensor.matmul(
                        pst[h][:, j, :],
                        mm(kt[h * D:(h + 1) * D, b, kc * P:(kc + 1) * P]),
                        mm(qt[h * D:(h + 1) * D, b, qb * QB:(qb + 1) * QB]),
                        start=True, stop=True)
                for h in range(HC):
                    if exp_eng is not None and g >= NG - EXP_NG:
                        exp_eng.tensor_scalar(
                            est[h].bitcast(INT16), pst[h], SCH_A, SCH_B,
                            mybir.AluOpType.mult, mybir.AluOpType.add)
                    else:
                        nc.scalar.activation(est[h], pst[h], EXP, scale=0.125)
                if g == 1:
                    flush_pending()
                # one-group software pipeline: PV of group g-1 runs after
                # scores of group g, hiding the exp latency from the PE
                def emit_pv(gg, est_=est):
                    for h in range(HC):
                        for j in range(KCG):
                            kc = gg * KCG + j
                            nc.tensor.matmul(
                                pa[h][:D + 1, :], mm(v1[:, b, h, kc, :]),
                                mm(est_[h][:, j, :]),
                                start=(kc == 0), stop=(kc == NKC - 1),
                                skip_group_check=True)
                PVD = int(os.environ.get("MHA_PVD", "4"))
                est_hist.append(est)
                if g >= PVD:
                    emit_pv(g - PVD, est_hist[-PVD - 1])
                if g == NG - 1:
                    for dd in range(PVD, 0, -1):
                        emit_pv(g - dd + 1, est_hist[-dd])
                    est_hist.clear()
            y_eng = {"pool": nc.gpsimd, "dve": nc.vector}[Y_ENG]
            for h in range(HC):
                rc = rc_pool.tile([1, QB], FP32, tag="rc",
                                  name=f"rc_{b}_{qb}_{h}")
                nc.vector.reciprocal(rc, pa[h][D:D + 1, :])
                bc = bc_pool.tile([D, QB], FP32, tag="bc",
                                  name=f"bc_{b}_{qb}_{h}")
                nc.gpsimd.partition_broadcast(bc, rc)
                nc.vector.tensor_tensor(
                    attnT[h * D:(h + 1) * D, b, qb * QB:(qb + 1) * QB],
                    pa[h][:D, :], bc, mybir.AluOpType.mult)

            def emit_outproj(b=b, qb=qb):
                for qc in range(QB // P):
                    q0 = qb * QB + qc * P
                    qci = qb * (QB // P) + qc
                    yt = None
                    if b == B - 1:
                        yt = y_pool.tile([P, E], ydt, tag="yt",
                                         name=f"yt_{b}_{qb}_{qc}")
                    for nh in range(2):
                        Y_POOL = os.environ.get("MHA_Y_POOL", "pa")
                        pool = {"pq": ps_q, "pa": ps_pa}[Y_POOL]
                        pst = pool.tile([P, 512], FP32, tag=Y_POOL,
                                        name=f"ps_y_{b}_{qb}_{qc}_{nh}")
                        nc.tensor.matmul(
                            pst, mm(attnT[:, b, q0:q0 + P]),
                            mm(wo_sb[:, b, nh * 512:(nh + 1) * 512]),
                            start=True, stop=True)
                        nsl = slice(nh * 512, (nh + 1) * 512)
                        if b == 0:
                            y_eng.tensor_copy(y_acc[:, qci, nsl], pst)
                        else:
                            nc.vector.tensor_tensor(
                                yt[:, nsl], pst, y_acc[:, qci, nsl],
                                mybir.AluOpType.add)
                    if b == B - 1:
                        nc.sync.dma_start(y[q0:q0 + P, :], yt)

            if LOOKAHEAD:
                pending.append(emit_outproj)
            else:
                emit_outproj()
    flush_pending()


def build_nc(mm_mode=MM_MODE, reps=1):
    sdt, QB = _mode_params(mm_mode)
    xdt = _xdt(mm_mode)
    ydt = mybir.dt.bfloat16 if YDT == "bf16" else FP32
    nc = bacc.Bacc("TRN2", target_bir_lowering=False, debug=False)
    if DP:
        xt = nc.dram_tensor("xt", [E, S], xdt, kind="ExternalInput").ap()
        wqkv = nc.dram_tensor("wqkv", [E, B * 3 * P], xdt,
                              kind="ExternalInput").ap()
        bqkv = nc.dram_tensor("bqkv", [B * 3 * P], FP32,
                              kind="ExternalInput").ap()
        wout = nc.dram_tensor("wout", [B * P, E], sdt,
                              kind="ExternalInput").ap()
        y = nc.dram_tensor("y", [S, E], ydt, kind="ExternalOutput").ap()
        builder = build_kernel_dp
    else:
        xt = nc.dram_tensor("xt", [E, R], xdt, kind="ExternalInput").ap()
        wqkv = nc.dram_tensor("wqkv", [E, 3 * P], xdt,
                              kind="ExternalInput").ap()
        bqkv = nc.dram_tensor("bqkv", [3 * P], FP32, kind="ExternalInput").ap()
        wout = nc.dram_tensor("wout", [P, E], sdt, kind="ExternalInput").ap()
        y = nc.dram_tensor("y", [R, E], ydt, kind="ExternalOutput").ap()
        builder = build_kernel
    with tile.TileContext(nc) as tc:
        for _ in range(reps):
            with ExitStack() as ctx:
                builder(tc, xt, wqkv, bqkv, wout, y, sdt, QB, mm_mode, ctx)
    nc.compile()
    return nc


def _round_f32r(x):
    """Round fp32 to the fp32r grid (11 explicit mantissa bits) the way the
    hardware expects matmul operands: add-half then truncate the low 12 bits."""
    bits = np.ascontiguousarray(x, np.float32).view(np.uint32)
    return (((bits + np.uint32(0x800)) & np.uint32(0xFFFFF000))
            .view(np.float32))


def shard_inputs(input_tensor, qkv_w, qkv_b, out_w, mm_mode=MM_MODE):
    """Build the 8 per-core input maps (numpy, host-side)."""
    sdt, _ = _mode_params(mm_mode)
    np_sdt = mybir.dt.np(sdt)
    np_xdt = mybir.dt.np(_xdt(mm_mode))

    def prep(a):
        a = np.ascontiguousarray(a).astype(np_sdt)
        return _round_f32r(a) if mm_mode == "f32r" else a

    def prep_x(a):
        a = np.ascontiguousarray(a, np.float32)
        if np_xdt == np.float32 and mm_mode == "f32r":
            return _round_f32r(a)
        return a.astype(np_xdt)

    X = np.asarray(input_tensor, np.float32).reshape(B, S, E)
    qkv_w = np.asarray(qkv_w, np.float32)
    qkv_b = np.asarray(qkv_b, np.float32)
    out_w = np.asarray(out_w, np.float32)
    in_maps = []
    if DP:
        # core c: batch c//4, heads (c%4)*4 .. +4 as two head-pair passes
        XT = {beta: prep_x(X[beta].T) for beta in range(B)}
        for c in range(NCORES):
            beta, gamma = c // 4, c % 4
            wq_p, bq_p, wo_p = [], [], []
            for pss in range(2):
                h0 = gamma * 4 + 2 * pss
                sl = slice(h0 * D, (h0 + 2) * D)       # 128 cols (2 heads)
                wq_p.append(np.concatenate(
                    [qkv_w[:, sl], qkv_w[:, E + h0 * D:E + (h0 + 2) * D],
                     qkv_w[:, 2 * E + h0 * D:2 * E + (h0 + 2) * D]], axis=1))
                bq_p.append(np.concatenate(
                    [qkv_b[sl], qkv_b[E + h0 * D:E + (h0 + 2) * D],
                     qkv_b[2 * E + h0 * D:2 * E + (h0 + 2) * D]]))
                wo_p.append(out_w[sl, :])
            in_maps.append({
                "xt": XT[beta],
                "wqkv": prep_x(np.concatenate(wq_p, axis=1)),
                "bqkv": np.ascontiguousarray(np.concatenate(bq_p)),
                "wout": prep(np.concatenate(wo_p, axis=0)),
            })
        return in_maps
    XT = prep_x(X.reshape(R, E).T)
    for c in range(NCORES):
        sl = slice(c * P, (c + 1) * P)
        wq = np.concatenate(
            [qkv_w[:, sl], qkv_w[:, E + c * P:E + (c + 1) * P],
             qkv_w[:, 2 * E + c * P:2 * E + (c + 1) * P]], axis=1)
        bq = np.concatenate(
            [qkv_b[sl], qkv_b[E + c * P:E + (c + 1) * P],
             qkv_b[2 * E + c * P:2 * E + (c + 1) * P]])
        in_maps.append({
            "xt": XT,
            "wqkv": prep_x(wq),
            "bqkv": np.ascontiguousarray(bq),
            "wout": prep(out_w[sl, :]),
        })
    return in_maps


_NC_CACHE = {}


def _get_nc(mm_mode):
    if mm_mode not in _NC_CACHE:
        _NC_CACHE[mm_mode] = build_nc(mm_mode)
    return _NC_CACHE[mm_mode]


LAST_RESULT = None


def kernel(input_tensor, qkv_w, qkv_b, out_w, out_b):
    global LAST_RESULT
    from concourse import bass_utils
    nc = _get_nc(MM_MODE)
    in_maps = shard_inputs(input_tensor, qkv_w, qkv_b, out_w, MM_MODE)
    res = bass_utils.run_bass_kernel_spmd(
        nc, in_maps, core_ids=list(range(NCORES)),
        trace=bool(int(os.environ.get("MHA_TRACE", "0"))))
    LAST_RESULT = res
    if DP:
        out = np.zeros((B, S, E), np.float32)
        for c, r in enumerate(res.results):
            out[c // 4] += np.asarray(r["y"], np.float32)
        out += np.asarray(out_b, np.float32)
        return out
    out = np.zeros((R, E), np.float32)
    for r in res.results:
        out += np.asarray(r["y"], np.float32)
    out += np.asarray(out_b, np.float32)
    return out.reshape(B, S, E)


def core_partial_ref(input_tensor, qkv_w, qkv_b, out_w, c):
    """Exact fp32 numpy reference for core c's partial output (for testing)."""
    X = np.asarray(input_tensor, np.float32).reshape(R, E)
    out = np.zeros((R, E), np.float32)
    for b in range(B):
        rows = slice(b * S, (b + 1) * S)
        for hl in range(HC):
            h = c * HC + hl
            q = X[rows] @ qkv_w[:, h * D:(h + 1) * D] + qkv_b[h * D:(h + 1) * D]
            k = X[rows] @ qkv_w[:, E + h * D:E + (h + 1) * D] + qkv_b[E + h * D:E + (h + 1) * D]
            v = X[rows] @ qkv_w[:, 2 * E + h * D:2 * E + (h + 1) * D] + qkv_b[2 * E + h * D:2 * E + (h + 1) * D]
            s = (q @ k.T) / np.sqrt(np.float32(D))
            p = np.exp(s - s.max(axis=1, keepdims=True))
            p /= p.sum(axis=1, keepdims=True)
            a = p @ v
            out[rows] += a @ out_w[h * D:(h + 1) * D, :]
    return out

